# revision 7
# baseline (speedup 1.0000x reference)
"""Additive attention (nn_AdditiveAttention) Bass kernel for 8 TRN2 NeuronCores.

Reference computation (B=16, Q=64, K=1024, QS=KS=VS=256, H=128):
    q = queries @ Wq                      # (B,Q,H)
    k = keys @ Wk                         # (B,K,H)
    feat = tanh(q[:,:,None,:] + k[:,None,:,:])   # (B,Q,K,H)
    scores = feat @ Ws                    # (B,Q,K)
    scores = where(arange(K) >= valid_len[b], scores, -1e6)
    out = softmax(scores) @ values        # (B,Q,VS)

Strategy: data-parallel over batch (2 batches per core), with
valid_len-aware skipping of fully-masked leading key blocks.

Per-core layout (one NeuronCore, batches "slot0" rows 0-63 / "slot1"
rows 64-127 of a 128-row (b,q) partition axis):
  - kfT[h, k] = (keys @ Wk).T computed by PE from host-pretransposed keysT.
  - tanh input per q built by DVE tensor_scalar add (kfT + qf[:,q]); ACT
    does tanh on QC queries per instruction (ACT is the bottleneck engine:
    1 elem/lane/cycle @ 1.2 GHz over B*Q*K_kept*H elements).
  - scores accumulate in PSUM via one matmul per (q, 512-col tile) with a
    shifted stationary matrix Z (Ws embedded in column 128), so row r of
    the [128 bq, 512] PSUM tile receives exactly that q's scores. The
    accumulation group is seeded by a mask matmul (identity @ additive
    mask) which also implements the valid_len masking.
  - softmax without max-subtraction (|scores| <= ~10): ACT exp PSUM->SBUF
    with fused row-sum (accum_out); DVE reciprocal + row scale.
  - attn @ values via PE transpose of the attention rows then one matmul
    per 128-key block against host-sliced values.
"""

import sys

if "/opt/trn_rl_repo" not in sys.path:
    sys.path.insert(0, "/opt/trn_rl_repo")

import numpy as np

import concourse.bass as bass  # noqa: F401  (engine types referenced via nc)
import concourse.mybir as mybir
import concourse.tile as tile
from concourse import bacc
from concourse.bass_utils import run_bass_kernel_spmd

LAST_RESULT = None  # BassKernelResults of the most recent kernel() call

B, Q, K = 16, 64, 1024
QS = KS = VS = 256
H = 128
NCORES = 8
NEG = -1.0e6
QC = 4  # queries per ACT tanh instruction (chunk)
F32 = mybir.dt.float32


def _build(L, k0, nblk):
    """Build the per-core Bass graph. L/k0/nblk are 2-element lists with the
    per-slot kept key length (multiple of 128), first kept key index, and
    number of kept 128-key blocks."""
    nc = bacc.Bacc("TRN2", target_bir_lowering=False, debug=False,
                   num_devices=NCORES)

    inp = {}
    for s in range(2):
        inp[f"keysT{s}"] = nc.dram_tensor(f"keysT{s}", [2, 128, L[s]], F32,
                                          kind="ExternalInput").ap()
        inp[f"queriesT{s}"] = nc.dram_tensor(f"queriesT{s}", [2, 128, Q], F32,
                                             kind="ExternalInput").ap()
        inp[f"values{s}"] = nc.dram_tensor(f"values{s}", [nblk[s], 128, VS], F32,
                                           kind="ExternalInput").ap()
    inp["maskm"] = nc.dram_tensor("maskm", [128, K], F32,
                                  kind="ExternalInput").ap()
    inp["Wk2"] = nc.dram_tensor("Wk2", [2, 128, H], F32,
                                kind="ExternalInput").ap()
    inp["Wq2"] = nc.dram_tensor("Wq2", [2, 128, H], F32,
                                kind="ExternalInput").ap()
    inp["ident"] = nc.dram_tensor("ident", [128, 128], F32,
                                  kind="ExternalInput").ap()
    inp["Zmat"] = nc.dram_tensor("Zmat", [128, 256], F32,
                                 kind="ExternalInput").ap()
    out_d = nc.dram_tensor("out", [128, VS], F32, kind="ExternalOutput").ap()

    with tile.TileContext(nc) as tc:
        with (
            tc.tile_pool(name="consts", bufs=1) as consts,
            tc.tile_pool(name="proj", bufs=1) as proj,
            tc.tile_pool(name="vals", bufs=1) as vals,
            tc.tile_pool(name="tanhbuf", bufs=2) as tanhbuf,
            tc.tile_pool(name="soft", bufs=1) as soft,
        ):
            ident_sb = consts.tile([128, 128], F32)
            nc.sync.dma_start(out=ident_sb, in_=inp["ident"])
            z_sb = consts.tile([128, 256], F32)
            nc.sync.dma_start(out=z_sb, in_=inp["Zmat"])
            maskm_sb = consts.tile([128, K], F32)
            nc.sync.dma_start(out=maskm_sb, in_=inp["maskm"])
            wk_sb = consts.tile([128, 2, H], F32)
            wq_sb = consts.tile([128, 2, H], F32)
            for c in range(2):
                nc.sync.dma_start(out=wk_sb[:, c, :], in_=inp["Wk2"][c])
                nc.sync.dma_start(out=wq_sb[:, c, :], in_=inp["Wq2"][c])

            # values (needed only at the end; DMA overlaps the main loop)
            vals_sb = []
            for s in range(2):
                v = vals.tile([128, nblk[s], VS], F32, name=f"vals{s}")
                for j in range(nblk[s]):
                    nc.sync.dma_start(out=v[:, j, :], in_=inp[f"values{s}"][j])
                vals_sb.append(v)

            # ---- phase 1: projections -------------------------------------
            kfT_sb = []
            qf_sb = []
            with tc.tile_pool(name="kin", bufs=1) as kin, \
                 tc.tile_pool(name="kfps", bufs=2, space="PSUM") as kfps:
                for s in range(2):
                    kT = kin.tile([128, 2, L[s]], F32, name=f"kT{s}", tag="kT")
                    for c in range(2):
                        nc.sync.dma_start(out=kT[:, c, :],
                                          in_=inp[f"keysT{s}"][c])
                    qT = kin.tile([128, 2, Q], F32, name=f"qT{s}", tag="qT")
                    for c in range(2):
                        nc.sync.dma_start(out=qT[:, c, :],
                                          in_=inp[f"queriesT{s}"][c])

                    kf = proj.tile([128, L[s]], F32, name=f"kfT{s}", tag=f"kf{s}")
                    for o in range(0, L[s], 512):
                        w = min(512, L[s] - o)
                        kf_ps = kfps.tile([128, 512], F32, tag="kfps")
                        nc.tensor.matmul(kf_ps[:, :w], wk_sb[:, 0, :],
                                         kT[:, 0, o:o + w], start=True,
                                         stop=False)
                        nc.tensor.matmul(kf_ps[:, :w], wk_sb[:, 1, :],
                                         kT[:, 1, o:o + w], start=False,
                                         stop=True)
                        nc.vector.tensor_copy(out=kf[:, o:o + w],
                                              in_=kf_ps[:, :w])
                    kfT_sb.append(kf)

                    qf_ps = kfps.tile([128, Q], F32, tag="qfps", bufs=1)
                    nc.tensor.matmul(qf_ps, wq_sb[:, 0, :], qT[:, 0, :],
                                     start=True, stop=False)
                    nc.tensor.matmul(qf_ps, wq_sb[:, 1, :], qT[:, 1, :],
                                     start=False, stop=True)
                    qf = proj.tile([128, Q], F32, name=f"qf{s}", tag=f"qf{s}")
                    nc.vector.tensor_copy(out=qf, in_=qf_ps)
                    qf_sb.append(qf)

            # ---- phase 2: scores ------------------------------------------
            scps = tc.alloc_tile_pool(name="scps", bufs=1, space="PSUM")
            trps = tc.alloc_tile_pool(name="trps", bufs=2, space="PSUM")
            ops = tc.alloc_tile_pool(name="ops", bufs=1, space="PSUM")
            scA = scps.tile([128, 512], F32, tag="scA")
            scB = scps.tile([128, 512], F32, tag="scB")
            nc.tensor.matmul(scA, ident_sb, maskm_sb[:, 0:512], start=True,
                             stop=False)
            nc.tensor.matmul(scB, ident_sb, maskm_sb[:, 512:1024], start=True,
                             stop=False)

            nchunks = Q // QC
            for s in range(2):
                Ls, k0s = L[s], k0[s]
                nA = 512 - k0s
                for c in range(nchunks):
                    tin = tanhbuf.tile([128, QC * Ls], F32, tag="tin")
                    for qi in range(QC):
                        q = c * QC + qi
                        nc.vector.tensor_scalar_add(
                            out=tin[:, qi * Ls:(qi + 1) * Ls],
                            in0=kfT_sb[s],
                            scalar1=qf_sb[s][:, q:q + 1])
                    tout = tanhbuf.tile([128, QC * Ls], F32, tag="tout")
                    nc.scalar.activation(out=tout, in_=tin,
                                         func=mybir.ActivationFunctionType.Tanh)
                    for qi in range(QC):
                        q = c * QC + qi
                        r = s * 64 + q
                        zw = z_sb[:, 128 - r:256 - r]
                        last = (s == 1) and (c == nchunks - 1) and (qi == QC - 1)
                        nc.tensor.matmul(scA[:, k0s:512], zw,
                                         tout[:, qi * Ls:qi * Ls + nA],
                                         start=False, stop=last)
                        nc.tensor.matmul(scB, zw,
                                         tout[:, qi * Ls + nA:qi * Ls + nA + 512],
                                         start=False, stop=last)

            # ---- phase 3: softmax -----------------------------------------
            expm = soft.tile([128, K], F32)
            sums = soft.tile([128, 2], F32)
            nc.scalar.activation(out=expm[:, 0:512], in_=scA,
                                 func=mybir.ActivationFunctionType.Exp,
                                 accum_out=sums[:, 0:1])
            nc.scalar.activation(out=expm[:, 512:1024], in_=scB,
                                 func=mybir.ActivationFunctionType.Exp,
                                 accum_out=sums[:, 1:2])
            stot = soft.tile([128, 1], F32)
            nc.vector.tensor_add(out=stot, in0=sums[:, 0:1], in1=sums[:, 1:2])
            rsum = soft.tile([128, 1], F32)
            nc.vector.reciprocal(out=rsum, in_=stot)
            nc.vector.tensor_scalar_mul(out=expm, in0=expm, scalar1=rsum)

            # ---- phase 4: attn @ values -----------------------------------
            jmin = min(k0) // 128
            PT = soft.tile([128, 8 - jmin, 128], F32)
            for jj, j in enumerate(range(jmin, 8)):
                tr_ps = trps.tile([128, 128], F32, tag="tr")
                nc.tensor.transpose(tr_ps, expm[:, j * 128:(j + 1) * 128],
                                    ident_sb)
                nc.vector.tensor_copy(out=PT[:, jj, :], in_=tr_ps)

            o_sb = soft.tile([128, VS], F32)
            for s in range(2):
                js = k0[s] // 128
                out_ps = ops.tile([64, VS], F32, tag=f"out{s}")
                for idx, j in enumerate(range(js, 8)):
                    nc.tensor.matmul(out_ps,
                                     PT[:, j - jmin, s * 64:s * 64 + 64],
                                     vals_sb[s][:, j - js, :],
                                     start=(idx == 0), stop=(j == 7))
                nc.vector.tensor_copy(out=o_sb[s * 64:(s + 1) * 64, :],
                                      in_=out_ps)
            nc.sync.dma_start(out=out_d, in_=o_sb)
            ops.release()
            trps.release()
            scps.release()

    nc.finalize()
    return nc


def kernel(queries, keys, values, valid_len, Wq, Wk, Ws):
    queries = np.ascontiguousarray(np.asarray(queries, dtype=np.float32))
    keys = np.ascontiguousarray(np.asarray(keys, dtype=np.float32))
    values = np.ascontiguousarray(np.asarray(values, dtype=np.float32))
    Wq = np.asarray(Wq, dtype=np.float32)
    Wk = np.asarray(Wk, dtype=np.float32)
    Ws = np.asarray(Ws, dtype=np.float32)
    vl = np.asarray(valid_len).astype(np.int64)
    assert queries.shape == (B, Q, QS) and keys.shape == (B, K, KS)
    assert values.shape == (B, K, VS) and vl.shape == (B,)

    # kept key range per batch: [k0b, K); first floor(vl/128) blocks are
    # entirely masked (never contribute) so we skip them. Keep >= 1 block.
    k0b = np.minimum(vl // 128, (K // 128) - 1).clip(0) * 128
    nbb = (K - k0b) // 128

    # Load balance across cores: slot0 takes the 8 largest workloads,
    # slot1 the 8 smallest. SPMD => per-slot work is the max over cores.
    order = np.argsort(-nbb, kind="stable")
    slots = [order[:NCORES], order[NCORES:]]
    nblk = [int(nbb[s].max()) for s in slots]
    L = [n * 128 for n in nblk]
    k0 = [K - l for l in L]

    nc = _build(L, k0, nblk)

    # host-side constants
    ident = np.eye(128, dtype=np.float32)
    Zmat = np.zeros((128, 256), dtype=np.float32)
    Zmat[:, 128] = Ws
    Wk2 = np.ascontiguousarray(Wk.reshape(2, 128, H))
    Wq2 = np.ascontiguousarray(Wq.reshape(2, 128, H))

    in_maps = []
    for core in range(NCORES):
        m = {"ident": ident, "Zmat": Zmat, "Wk2": Wk2, "Wq2": Wq2}
        maskm = np.zeros((128, K), dtype=np.float32)
        for s in range(2):
            b = int(slots[s][core])
            m[f"keysT{s}"] = np.ascontiguousarray(
                keys[b, k0[s]:, :].T.reshape(2, 128, L[s]))
            m[f"queriesT{s}"] = np.ascontiguousarray(
                queries[b].T.reshape(2, 128, Q))
            m[f"values{s}"] = np.ascontiguousarray(
                values[b, k0[s]:, :].reshape(nblk[s], 128, VS))
            maskm[s * 64:(s + 1) * 64, :int(vl[b])] = NEG
        m["maskm"] = maskm
        in_maps.append(m)

    res = run_bass_kernel_spmd(nc, in_maps, core_ids=list(range(NCORES)),
                               trace=False)
    global LAST_RESULT
    LAST_RESULT = res

    out = np.empty((B, Q, VS), dtype=np.float32)
    for core in range(NCORES):
        o = res.results[core]["out"]  # [128, VS]
        for s in range(2):
            b = int(slots[s][core])
            out[b] = o[s * 64:(s + 1) * 64, :]
    return out


# revision 9
# speedup vs baseline: 1.6999x; 1.6999x over previous
"""Additive attention (nn_AdditiveAttention) Bass kernel for 8 TRN2 NeuronCores.

Reference computation (B=16, Q=64, K=1024, QS=KS=VS=256, H=128):
    q = queries @ Wq                      # (B,Q,H)
    k = keys @ Wk                         # (B,K,H)
    feat = tanh(q[:,:,None,:] + k[:,None,:,:])   # (B,Q,K,H)
    scores = feat @ Ws                    # (B,Q,K)
    scores = where(arange(K) >= valid_len[b], scores, -1e6)
    out = softmax(scores) @ values        # (B,Q,VS)

Strategy: data-parallel over batch (2 batches per core, "slot0" rows 0-63
and "slot1" rows 64-127 of a 128-row (b,q) partition axis), with
valid_len-aware skipping of fully-masked leading key blocks and bf16
compute on the PE/DVE-heavy stages (fp32 PE matmul runs at half rate).

Per-core pipeline:
  - kfT[h, k] = (keys @ Wk).T from host-pretransposed bf16 keysT (PE).
  - per q: DVE tensor_scalar add (kfT + qf[:,q]) in bf16 (4x mode); ACT
    tanh on QC queries per instruction (ACT is the floor engine:
    1 elem/lane/cycle @ 1.2 GHz over B*Q*K_kept*H elements).
  - scores accumulate in PSUM fp32 via one bf16 matmul per (q, col tile)
    with a shifted stationary matrix Z (Ws embedded in column 128): row r
    of the [128 bq, 512] PSUM tile receives exactly q=r's scores. The
    accumulation group is seeded by an fp32 mask matmul (identity @
    additive -1e6 mask) implementing the valid_len masking.
  - softmax without max-subtraction (|scores| <= ~10): ACT exp PSUM->SBUF
    with fused row-sum (accum_out); DVE reciprocal + row scale.
  - attn @ values: PE transpose of attention rows, then one bf16 matmul
    per 128-key block against host-sliced bf16 values.
"""

import sys

if "/opt/trn_rl_repo" not in sys.path:
    sys.path.insert(0, "/opt/trn_rl_repo")

import ml_dtypes
import numpy as np

import concourse.bass as bass  # noqa: F401
import concourse.mybir as mybir
import concourse.tile as tile
from concourse import bacc
from concourse.bass_utils import run_bass_kernel_spmd

LAST_RESULT = None  # BassKernelResults of the most recent kernel() call

B, Q, K = 16, 64, 1024
QS = KS = VS = 256
H = 128
NCORES = 8
NEG = -1.0e6
QC = 8  # queries per ACT tanh instruction (chunk)
F32 = mybir.dt.float32
BF16 = mybir.dt.bfloat16
NP_BF16 = ml_dtypes.bfloat16


def _build(L, k0, nblk):
    """Build the per-core Bass graph. L/k0/nblk are 2-element lists with the
    per-slot kept key length (multiple of 128), first kept key index, and
    number of kept 128-key blocks."""
    nc = bacc.Bacc("TRN2", target_bir_lowering=False, debug=False,
                   num_devices=NCORES)

    inp = {}
    for s in range(2):
        inp[f"keysT{s}"] = nc.dram_tensor(f"keysT{s}", [2, 128, L[s]], BF16,
                                          kind="ExternalInput").ap()
        inp[f"queriesT{s}"] = nc.dram_tensor(f"queriesT{s}", [2, 128, Q], BF16,
                                             kind="ExternalInput").ap()
        inp[f"values{s}"] = nc.dram_tensor(f"values{s}", [nblk[s], 128, VS],
                                           BF16, kind="ExternalInput").ap()
    inp["maskm"] = nc.dram_tensor("maskm", [128, K], F32,
                                  kind="ExternalInput").ap()
    inp["Wk2"] = nc.dram_tensor("Wk2", [2, 128, H], BF16,
                                kind="ExternalInput").ap()
    inp["Wq2"] = nc.dram_tensor("Wq2", [2, 128, H], BF16,
                                kind="ExternalInput").ap()
    inp["ident"] = nc.dram_tensor("ident", [128, 128], F32,
                                  kind="ExternalInput").ap()
    inp["Zmat"] = nc.dram_tensor("Zmat", [128, 256], BF16,
                                 kind="ExternalInput").ap()
    out_d = nc.dram_tensor("out", [128, VS], F32, kind="ExternalOutput").ap()

    with tile.TileContext(nc) as tc:
        with (
            tc.tile_pool(name="consts", bufs=1) as consts,
            tc.tile_pool(name="proj", bufs=1) as proj,
            tc.tile_pool(name="vals", bufs=1) as vals,
            tc.tile_pool(name="tanhbuf", bufs=2) as tanhbuf,
            tc.tile_pool(name="soft", bufs=1) as soft,
        ):
            ident_sb = consts.tile([128, 128], F32)
            nc.sync.dma_start(out=ident_sb, in_=inp["ident"])
            z_sb = consts.tile([128, 256], BF16)
            nc.sync.dma_start(out=z_sb, in_=inp["Zmat"])
            maskm_sb = consts.tile([128, K], F32)
            nc.sync.dma_start(out=maskm_sb, in_=inp["maskm"])
            wk_sb = consts.tile([128, 2, H], BF16)
            wq_sb = consts.tile([128, 2, H], BF16)
            for c in range(2):
                nc.sync.dma_start(out=wk_sb[:, c, :], in_=inp["Wk2"][c])
                nc.sync.dma_start(out=wq_sb[:, c, :], in_=inp["Wq2"][c])

            # values (needed only at the end; DMA overlaps the main loop)
            vals_sb = []
            for s in range(2):
                v = vals.tile([128, nblk[s], VS], BF16, name=f"vals{s}")
                for j in range(nblk[s]):
                    nc.sync.dma_start(out=v[:, j, :], in_=inp[f"values{s}"][j])
                vals_sb.append(v)

            # ---- phase 1: projections -------------------------------------
            kfT_sb = []
            qf_sb = []
            with tc.tile_pool(name="kin", bufs=1) as kin, \
                 tc.tile_pool(name="kfps", bufs=2, space="PSUM") as kfps:
                for s in range(2):
                    kT = kin.tile([128, 2, L[s]], BF16, name=f"kT{s}", tag="kT")
                    for c in range(2):
                        nc.sync.dma_start(out=kT[:, c, :],
                                          in_=inp[f"keysT{s}"][c])
                    qT = kin.tile([128, 2, Q], BF16, name=f"qT{s}", tag="qT")
                    for c in range(2):
                        nc.sync.dma_start(out=qT[:, c, :],
                                          in_=inp[f"queriesT{s}"][c])

                    kf = proj.tile([128, L[s]], BF16, name=f"kfT{s}",
                                   tag=f"kf{s}")
                    for o in range(0, L[s], 512):
                        w = min(512, L[s] - o)
                        kf_ps = kfps.tile([128, 512], F32, tag="kfps")
                        nc.tensor.matmul(kf_ps[:, :w], wk_sb[:, 0, :],
                                         kT[:, 0, o:o + w], start=True,
                                         stop=False)
                        nc.tensor.matmul(kf_ps[:, :w], wk_sb[:, 1, :],
                                         kT[:, 1, o:o + w], start=False,
                                         stop=True)
                        nc.vector.tensor_copy(out=kf[:, o:o + w],
                                              in_=kf_ps[:, :w])
                    kfT_sb.append(kf)

                    qf_ps = kfps.tile([128, Q], F32, tag="qfps", bufs=1)
                    nc.tensor.matmul(qf_ps, wq_sb[:, 0, :], qT[:, 0, :],
                                     start=True, stop=False)
                    nc.tensor.matmul(qf_ps, wq_sb[:, 1, :], qT[:, 1, :],
                                     start=False, stop=True)
                    qf = proj.tile([128, Q], F32, name=f"qf{s}", tag=f"qf{s}")
                    nc.vector.tensor_copy(out=qf, in_=qf_ps)
                    qf_sb.append(qf)

            # ---- phase 2: scores ------------------------------------------
            scps = tc.alloc_tile_pool(name="scps", bufs=1, space="PSUM")
            trps = tc.alloc_tile_pool(name="trps", bufs=2, space="PSUM")
            ops = tc.alloc_tile_pool(name="ops", bufs=1, space="PSUM")
            scA = scps.tile([128, 512], F32, tag="scA")
            scB = scps.tile([128, 512], F32, tag="scB")
            nc.tensor.matmul(scA, ident_sb, maskm_sb[:, 0:512], start=True,
                             stop=False)
            nc.tensor.matmul(scB, ident_sb, maskm_sb[:, 512:1024], start=True,
                             stop=False)

            nchunks = Q // QC
            for s in range(2):
                Ls, k0s = L[s], k0[s]
                nA = 512 - k0s
                for c in range(nchunks):
                    tin = tanhbuf.tile([128, QC * Ls], BF16, tag="tin")
                    for qi in range(QC):
                        q = c * QC + qi
                        nc.vector.tensor_scalar_add(
                            out=tin[:, qi * Ls:(qi + 1) * Ls],
                            in0=kfT_sb[s],
                            scalar1=qf_sb[s][:, q:q + 1])
                    tout = tanhbuf.tile([128, QC * Ls], BF16, tag="tout")
                    nc.scalar.activation(out=tout, in_=tin,
                                         func=mybir.ActivationFunctionType.Tanh)
                    for qi in range(QC):
                        q = c * QC + qi
                        r = s * 64 + q
                        zw = z_sb[:, 128 - r:256 - r]
                        last = (s == 1) and (c == nchunks - 1) and (qi == QC - 1)
                        nc.tensor.matmul(scA[:, k0s:512], zw,
                                         tout[:, qi * Ls:qi * Ls + nA],
                                         start=False, stop=last)
                        nc.tensor.matmul(scB, zw,
                                         tout[:, qi * Ls + nA:qi * Ls + nA + 512],
                                         start=False, stop=last)

            # ---- phase 3: softmax -----------------------------------------
            expm = soft.tile([128, K], F32)
            sums = soft.tile([128, 2], F32)
            nc.scalar.activation(out=expm[:, 0:512], in_=scA,
                                 func=mybir.ActivationFunctionType.Exp,
                                 accum_out=sums[:, 0:1])
            nc.scalar.activation(out=expm[:, 512:1024], in_=scB,
                                 func=mybir.ActivationFunctionType.Exp,
                                 accum_out=sums[:, 1:2])
            stot = soft.tile([128, 1], F32)
            nc.vector.tensor_add(out=stot, in0=sums[:, 0:1], in1=sums[:, 1:2])
            rsum = soft.tile([128, 1], F32)
            nc.vector.reciprocal(out=rsum, in_=stot)
            nc.vector.tensor_scalar_mul(out=expm, in0=expm, scalar1=rsum)

            # ---- phase 4: attn @ values -----------------------------------
            jmin = min(k0) // 128
            PT = soft.tile([128, 8 - jmin, 128], BF16)
            for jj, j in enumerate(range(jmin, 8)):
                tr_ps = trps.tile([128, 128], F32, tag="tr")
                nc.tensor.transpose(tr_ps, expm[:, j * 128:(j + 1) * 128],
                                    ident_sb)
                nc.vector.tensor_copy(out=PT[:, jj, :], in_=tr_ps)

            o_sb = soft.tile([128, VS], F32)
            for s in range(2):
                js = k0[s] // 128
                out_ps = ops.tile([64, VS], F32, tag=f"out{s}")
                for idx, j in enumerate(range(js, 8)):
                    nc.tensor.matmul(out_ps,
                                     PT[:, j - jmin, s * 64:s * 64 + 64],
                                     vals_sb[s][:, j - js, :],
                                     start=(idx == 0), stop=(j == 7))
                nc.vector.tensor_copy(out=o_sb[s * 64:(s + 1) * 64, :],
                                      in_=out_ps)
            nc.sync.dma_start(out=out_d, in_=o_sb)
            ops.release()
            trps.release()
            scps.release()

    nc.finalize()
    return nc


def kernel(queries, keys, values, valid_len, Wq, Wk, Ws):
    queries = np.asarray(queries, dtype=np.float32)
    keys = np.asarray(keys, dtype=np.float32)
    values = np.asarray(values, dtype=np.float32)
    Wq = np.asarray(Wq, dtype=np.float32)
    Wk = np.asarray(Wk, dtype=np.float32)
    Ws = np.asarray(Ws, dtype=np.float32)
    vl = np.asarray(valid_len).astype(np.int64)
    assert queries.shape == (B, Q, QS) and keys.shape == (B, K, KS)
    assert values.shape == (B, K, VS) and vl.shape == (B,)

    # kept key range per batch: [k0b, K); the first floor(vl/128) blocks are
    # entirely masked (never contribute) so we skip them. Keep >= 1 block.
    k0b = np.minimum(vl // 128, (K // 128) - 1).clip(0) * 128
    nbb = (K - k0b) // 128

    # Load balance across cores: slot0 takes the 8 largest workloads,
    # slot1 the 8 smallest. SPMD => per-slot work is the max over cores.
    order = np.argsort(-nbb, kind="stable")
    slots = [order[:NCORES], order[NCORES:]]
    nblk = [int(nbb[s].max()) for s in slots]
    L = [n * 128 for n in nblk]
    k0 = [K - l for l in L]

    nc = _build(L, k0, nblk)

    # host-side constants
    ident = np.eye(128, dtype=np.float32)
    Zmat = np.zeros((128, 256), dtype=NP_BF16)
    Zmat[:, 128] = Ws.astype(NP_BF16)
    Wk2 = np.ascontiguousarray(Wk.reshape(2, 128, H).astype(NP_BF16))
    Wq2 = np.ascontiguousarray(Wq.reshape(2, 128, H).astype(NP_BF16))

    in_maps = []
    for core in range(NCORES):
        m = {"ident": ident, "Zmat": Zmat, "Wk2": Wk2, "Wq2": Wq2}
        maskm = np.zeros((128, K), dtype=np.float32)
        for s in range(2):
            b = int(slots[s][core])
            m[f"keysT{s}"] = np.ascontiguousarray(
                keys[b, k0[s]:, :].T.reshape(2, 128, L[s]).astype(NP_BF16))
            m[f"queriesT{s}"] = np.ascontiguousarray(
                queries[b].T.reshape(2, 128, Q).astype(NP_BF16))
            m[f"values{s}"] = np.ascontiguousarray(
                values[b, k0[s]:, :].reshape(nblk[s], 128, VS).astype(NP_BF16))
            maskm[s * 64:(s + 1) * 64, :int(vl[b])] = NEG
        m["maskm"] = maskm
        in_maps.append(m)

    res = run_bass_kernel_spmd(nc, in_maps, core_ids=list(range(NCORES)),
                               trace=False)
    global LAST_RESULT
    LAST_RESULT = res

    out = np.empty((B, Q, VS), dtype=np.float32)
    for core in range(NCORES):
        o = res.results[core]["out"]  # [128, VS]
        for s in range(2):
            b = int(slots[s][core])
            out[b] = o[s * 64:(s + 1) * 64, :]
    return out


# revision 11
# speedup vs baseline: 1.8149x; 1.0677x over previous
"""Additive attention (nn_AdditiveAttention) Bass kernel for 8 TRN2 NeuronCores.

Reference computation (B=16, Q=64, K=1024, QS=KS=VS=256, H=128):
    q = queries @ Wq                      # (B,Q,H)
    k = keys @ Wk                         # (B,K,H)
    feat = tanh(q[:,:,None,:] + k[:,None,:,:])   # (B,Q,K,H)
    scores = feat @ Ws                    # (B,Q,K)
    scores = where(arange(K) >= valid_len[b], scores, -1e6)
    out = softmax(scores) @ values        # (B,Q,VS)

Strategy: data-parallel over batch (2 batches per core, "slot0" rows 0-63
and "slot1" rows 64-127 of a 128-row (b,q) partition axis), with
valid_len-aware skipping of fully-masked leading key blocks and bf16
compute on the PE/DVE-heavy stages (fp32 PE matmul runs at half rate).

Per-core pipeline:
  - kfT[h, k] = (keys @ Wk).T from host-pretransposed bf16 keysT (PE).
  - per q: DVE tensor_scalar add (kfT + qf[:,q]) in bf16 (4x mode); ACT
    tanh on QC queries per instruction (ACT is the floor engine:
    1 elem/lane/cycle @ 1.2 GHz over B*Q*K_kept*H elements).
  - scores accumulate in PSUM fp32 via one bf16 matmul per (q, col tile)
    with a shifted stationary matrix Z (Ws embedded in column 128): row r
    of the [128 bq, 512] PSUM tile receives exactly q=r's scores. The
    accumulation group is seeded by an fp32 mask matmul (identity @
    additive -1e6 mask) implementing the valid_len masking.
  - softmax without max-subtraction (|scores| <= ~10): ACT exp PSUM->SBUF
    with fused row-sum (accum_out); DVE reciprocal + row scale.
  - attn @ values: PE transpose of attention rows, then one bf16 matmul
    per 128-key block against host-sliced bf16 values.
"""

import sys

if "/opt/trn_rl_repo" not in sys.path:
    sys.path.insert(0, "/opt/trn_rl_repo")

import ml_dtypes
import numpy as np

import concourse.bass as bass  # noqa: F401
import concourse.mybir as mybir
import concourse.tile as tile
from concourse import bacc
from concourse.bass_utils import run_bass_kernel_spmd

LAST_RESULT = None  # BassKernelResults of the most recent kernel() call

B, Q, K = 16, 64, 1024
QS = KS = VS = 256
H = 128
NCORES = 8
NEG = -1.0e6
QC = 8  # queries per ACT tanh instruction (chunk)
F32 = mybir.dt.float32
BF16 = mybir.dt.bfloat16
NP_BF16 = ml_dtypes.bfloat16


def _build(L, k0, nblk):
    """Build the per-core Bass graph. L/k0/nblk are 2-element lists with the
    per-slot kept key length (multiple of 128), first kept key index, and
    number of kept 128-key blocks."""
    nc = bacc.Bacc("TRN2", target_bir_lowering=False, debug=False,
                   num_devices=NCORES)

    inp = {}
    for s in range(2):
        inp[f"keysT{s}"] = nc.dram_tensor(f"keysT{s}", [2, 128, L[s]], BF16,
                                          kind="ExternalInput").ap()
        inp[f"queriesT{s}"] = nc.dram_tensor(f"queriesT{s}", [2, 128, Q], BF16,
                                             kind="ExternalInput").ap()
        inp[f"values{s}"] = nc.dram_tensor(f"values{s}", [nblk[s], 128, VS],
                                           BF16, kind="ExternalInput").ap()
    inp["maskm"] = nc.dram_tensor("maskm", [128, K], F32,
                                  kind="ExternalInput").ap()
    inp["Wk2"] = nc.dram_tensor("Wk2", [2, 128, H], BF16,
                                kind="ExternalInput").ap()
    inp["Wq2"] = nc.dram_tensor("Wq2", [2, 128, H], BF16,
                                kind="ExternalInput").ap()
    inp["ident"] = nc.dram_tensor("ident", [128, 128], F32,
                                  kind="ExternalInput").ap()
    inp["Zmat"] = nc.dram_tensor("Zmat", [128, 256], BF16,
                                 kind="ExternalInput").ap()
    out_d = nc.dram_tensor("out", [128, VS], F32, kind="ExternalOutput").ap()

    with tile.TileContext(nc) as tc:
        with (
            tc.tile_pool(name="consts", bufs=1) as consts,
            tc.tile_pool(name="proj", bufs=1) as proj,
            tc.tile_pool(name="vals", bufs=1) as vals,
            tc.tile_pool(name="tanhbuf", bufs=3) as tanhbuf,
            tc.tile_pool(name="soft", bufs=1) as soft,
        ):
            ident_sb = consts.tile([128, 128], F32)
            nc.sync.dma_start(out=ident_sb, in_=inp["ident"])
            z_sb = consts.tile([128, 256], BF16)
            nc.sync.dma_start(out=z_sb, in_=inp["Zmat"])
            maskm_sb = consts.tile([128, K], F32)
            nc.sync.dma_start(out=maskm_sb, in_=inp["maskm"])
            wk_sb = consts.tile([128, 2, H], BF16)
            wq_sb = consts.tile([128, 2, H], BF16)
            for c in range(2):
                nc.sync.dma_start(out=wk_sb[:, c, :], in_=inp["Wk2"][c])
                nc.sync.dma_start(out=wq_sb[:, c, :], in_=inp["Wq2"][c])

            # ---- phase 1: projections -------------------------------------
            # keysT DMAs are chunked per 512 columns so the kproj matmuls
            # (and hence the first tanh) start as early as possible; values
            # loads are issued after (only needed in the tail).
            kfT_sb = []
            qf_sb = []
            with tc.tile_pool(name="kin", bufs=1) as kin, \
                 tc.tile_pool(name="kfps", bufs=2, space="PSUM") as kfps:
                for s in range(2):
                    kT = kin.tile([128, 2, L[s]], BF16, name=f"kT{s}", tag="kT")
                    qT = kin.tile([128, 2, Q], BF16, name=f"qT{s}", tag="qT")
                    for c in range(2):
                        nc.sync.dma_start(out=qT[:, c, :],
                                          in_=inp[f"queriesT{s}"][c])
                    for o in range(0, L[s], 512):
                        w = min(512, L[s] - o)
                        for c in range(2):
                            nc.sync.dma_start(
                                out=kT[:, c, o:o + w],
                                in_=inp[f"keysT{s}"][c, :, o:o + w])

                    kf = proj.tile([128, L[s]], BF16, name=f"kfT{s}",
                                   tag=f"kf{s}")
                    for o in range(0, L[s], 512):
                        w = min(512, L[s] - o)
                        kf_ps = kfps.tile([128, 512], F32, tag="kfps")
                        nc.tensor.matmul(kf_ps[:, :w], wk_sb[:, 0, :],
                                         kT[:, 0, o:o + w], start=True,
                                         stop=False)
                        nc.tensor.matmul(kf_ps[:, :w], wk_sb[:, 1, :],
                                         kT[:, 1, o:o + w], start=False,
                                         stop=True)
                        nc.vector.tensor_copy(out=kf[:, o:o + w],
                                              in_=kf_ps[:, :w])
                    kfT_sb.append(kf)

                    qf_ps = kfps.tile([128, Q], F32, tag="qfps", bufs=1)
                    nc.tensor.matmul(qf_ps, wq_sb[:, 0, :], qT[:, 0, :],
                                     start=True, stop=False)
                    nc.tensor.matmul(qf_ps, wq_sb[:, 1, :], qT[:, 1, :],
                                     start=False, stop=True)
                    qf = proj.tile([128, Q], F32, name=f"qf{s}", tag=f"qf{s}")
                    nc.vector.tensor_copy(out=qf, in_=qf_ps)
                    qf_sb.append(qf)

            # values (needed only at the end; DMA overlaps the main loop)
            vals_sb = []
            for s in range(2):
                v = vals.tile([128, nblk[s], VS], BF16, name=f"vals{s}")
                for j in range(nblk[s]):
                    nc.sync.dma_start(out=v[:, j, :], in_=inp[f"values{s}"][j])
                vals_sb.append(v)

            # ---- phase 2: scores ------------------------------------------
            scps = tc.alloc_tile_pool(name="scps", bufs=1, space="PSUM")
            trps = tc.alloc_tile_pool(name="trps", bufs=2, space="PSUM")
            ops = tc.alloc_tile_pool(name="ops", bufs=1, space="PSUM")
            scA = scps.tile([128, 512], F32, tag="scA")
            scB = scps.tile([128, 512], F32, tag="scB")
            nc.tensor.matmul(scA, ident_sb, maskm_sb[:, 0:512], start=True,
                             stop=False)
            nc.tensor.matmul(scB, ident_sb, maskm_sb[:, 512:1024], start=True,
                             stop=False)

            nchunks = Q // QC
            for s in range(2):
                Ls, k0s = L[s], k0[s]
                nA = 512 - k0s
                for c in range(nchunks):
                    tin = tanhbuf.tile([128, QC * Ls], BF16, tag="tin")
                    for qi in range(QC):
                        q = c * QC + qi
                        nc.vector.tensor_scalar_add(
                            out=tin[:, qi * Ls:(qi + 1) * Ls],
                            in0=kfT_sb[s],
                            scalar1=qf_sb[s][:, q:q + 1])
                    tout = tanhbuf.tile([128, QC * Ls], BF16, tag="tout")
                    nc.scalar.activation(out=tout, in_=tin,
                                         func=mybir.ActivationFunctionType.Tanh)
                    for qi in range(QC):
                        q = c * QC + qi
                        r = s * 64 + q
                        zw = z_sb[:, 128 - r:256 - r]
                        last = (s == 1) and (c == nchunks - 1) and (qi == QC - 1)
                        nc.tensor.matmul(scA[:, k0s:512], zw,
                                         tout[:, qi * Ls:qi * Ls + nA],
                                         start=False, stop=last)
                        nc.tensor.matmul(scB, zw,
                                         tout[:, qi * Ls + nA:qi * Ls + nA + 512],
                                         start=False, stop=last)

            # ---- phase 3: softmax -----------------------------------------
            expm = soft.tile([128, K], F32)
            sums = soft.tile([128, 2], F32)
            nc.scalar.activation(out=expm[:, 0:512], in_=scA,
                                 func=mybir.ActivationFunctionType.Exp,
                                 accum_out=sums[:, 0:1])
            nc.scalar.activation(out=expm[:, 512:1024], in_=scB,
                                 func=mybir.ActivationFunctionType.Exp,
                                 accum_out=sums[:, 1:2])
            stot = soft.tile([128, 1], F32)
            nc.vector.tensor_add(out=stot, in0=sums[:, 0:1], in1=sums[:, 1:2])
            rsum = soft.tile([128, 1], F32)
            nc.vector.reciprocal(out=rsum, in_=stot)
            nc.vector.tensor_scalar_mul(out=expm, in0=expm, scalar1=rsum)

            # ---- phase 4: attn @ values -----------------------------------
            jmin = min(k0) // 128
            PT = soft.tile([128, 8 - jmin, 128], BF16)
            for jj, j in enumerate(range(jmin, 8)):
                tr_ps = trps.tile([128, 128], F32, tag="tr")
                nc.tensor.transpose(tr_ps, expm[:, j * 128:(j + 1) * 128],
                                    ident_sb)
                nc.vector.tensor_copy(out=PT[:, jj, :], in_=tr_ps)

            o_sb = soft.tile([128, VS], F32)
            for s in range(2):
                js = k0[s] // 128
                out_ps = ops.tile([64, VS], F32, tag=f"out{s}")
                for idx, j in enumerate(range(js, 8)):
                    nc.tensor.matmul(out_ps,
                                     PT[:, j - jmin, s * 64:s * 64 + 64],
                                     vals_sb[s][:, j - js, :],
                                     start=(idx == 0), stop=(j == 7))
                nc.vector.tensor_copy(out=o_sb[s * 64:(s + 1) * 64, :],
                                      in_=out_ps)
            nc.sync.dma_start(out=out_d, in_=o_sb)
            ops.release()
            trps.release()
            scps.release()

    nc.finalize()
    return nc


def kernel(queries, keys, values, valid_len, Wq, Wk, Ws):
    queries = np.asarray(queries, dtype=np.float32)
    keys = np.asarray(keys, dtype=np.float32)
    values = np.asarray(values, dtype=np.float32)
    Wq = np.asarray(Wq, dtype=np.float32)
    Wk = np.asarray(Wk, dtype=np.float32)
    Ws = np.asarray(Ws, dtype=np.float32)
    vl = np.asarray(valid_len).astype(np.int64)
    assert queries.shape == (B, Q, QS) and keys.shape == (B, K, KS)
    assert values.shape == (B, K, VS) and vl.shape == (B,)

    # kept key range per batch: [k0b, K); the first floor(vl/128) blocks are
    # entirely masked (never contribute) so we skip them. Keep >= 1 block.
    k0b = np.minimum(vl // 128, (K // 128) - 1).clip(0) * 128
    nbb = (K - k0b) // 128

    # Load balance across cores: slot0 takes the 8 largest workloads,
    # slot1 the 8 smallest. SPMD => per-slot work is the max over cores.
    order = np.argsort(-nbb, kind="stable")
    slots = [order[:NCORES], order[NCORES:]]
    nblk = [int(nbb[s].max()) for s in slots]
    L = [n * 128 for n in nblk]
    k0 = [K - l for l in L]

    nc = _build(L, k0, nblk)

    # host-side constants
    ident = np.eye(128, dtype=np.float32)
    Zmat = np.zeros((128, 256), dtype=NP_BF16)
    Zmat[:, 128] = Ws.astype(NP_BF16)
    Wk2 = np.ascontiguousarray(Wk.reshape(2, 128, H).astype(NP_BF16))
    Wq2 = np.ascontiguousarray(Wq.reshape(2, 128, H).astype(NP_BF16))

    in_maps = []
    for core in range(NCORES):
        m = {"ident": ident, "Zmat": Zmat, "Wk2": Wk2, "Wq2": Wq2}
        maskm = np.zeros((128, K), dtype=np.float32)
        for s in range(2):
            b = int(slots[s][core])
            m[f"keysT{s}"] = np.ascontiguousarray(
                keys[b, k0[s]:, :].T.reshape(2, 128, L[s]).astype(NP_BF16))
            m[f"queriesT{s}"] = np.ascontiguousarray(
                queries[b].T.reshape(2, 128, Q).astype(NP_BF16))
            m[f"values{s}"] = np.ascontiguousarray(
                values[b, k0[s]:, :].reshape(nblk[s], 128, VS).astype(NP_BF16))
            maskm[s * 64:(s + 1) * 64, :int(vl[b])] = NEG
        m["maskm"] = maskm
        in_maps.append(m)

    res = run_bass_kernel_spmd(nc, in_maps, core_ids=list(range(NCORES)),
                               trace=False)
    global LAST_RESULT
    LAST_RESULT = res

    out = np.empty((B, Q, VS), dtype=np.float32)
    for core in range(NCORES):
        o = res.results[core]["out"]  # [128, VS]
        for s in range(2):
            b = int(slots[s][core])
            out[b] = o[s * 64:(s + 1) * 64, :]
    return out


# revision 13
# speedup vs baseline: 1.9086x; 1.0516x over previous
"""Additive attention (nn_AdditiveAttention) Bass kernel for 8 TRN2 NeuronCores.

Reference computation (B=16, Q=64, K=1024, QS=KS=VS=256, H=128):
    q = queries @ Wq                      # (B,Q,H)
    k = keys @ Wk                         # (B,K,H)
    feat = tanh(q[:,:,None,:] + k[:,None,:,:])   # (B,Q,K,H)
    scores = feat @ Ws                    # (B,Q,K)
    scores = where(arange(K) >= valid_len[b], scores, -1e6)
    out = softmax(scores) @ values        # (B,Q,VS)

Strategy: data-parallel over batch (2 batches per core, "slot0" rows 0-63
and "slot1" rows 64-127 of a 128-row (b,q) partition axis), with
valid_len-aware skipping of masked leading keys (k0 = min valid_len over
the slot, rounded to 8) and bf16 compute on the PE/DVE-heavy stages
(fp32 PE matmul runs at half rate).

Per-core pipeline:
  - kfT[h, k] = (keys @ Wk).T from host-pretransposed bf16 keysT (PE).
  - per q: DVE tensor_scalar add (kfT + qf[:,q]) in bf16 (4x mode); ACT
    tanh on QC queries per instruction (ACT is the floor engine:
    1 elem/lane/cycle @ 1.2 GHz over B*Q*K_kept*H elements).
  - scores accumulate in PSUM fp32 via one bf16 matmul per (q, col tile)
    with a shifted stationary matrix Z (Ws embedded in column 128): row r
    of the [128 bq, 512] PSUM tile receives exactly q=r's scores. The
    accumulation group is seeded by an fp32 mask matmul (identity @
    additive -1e6 mask) implementing the valid_len masking.
  - softmax without max-subtraction (|scores| <= ~10): ACT exp PSUM->SBUF
    with fused row-sum (accum_out). Attention rows are transposed
    UNNORMALIZED (PE transpose per 128-key block, right after the exp of
    that half); normalization is applied to the final [64, VS] outputs
    (slot1's 1/sum vector is moved to partitions 0-63 by a tiny
    SBUF->SBUF DMA).
  - attn @ values: one bf16 matmul per 128-key block against host-sliced
    bf16 values.
"""

import sys

if "/opt/trn_rl_repo" not in sys.path:
    sys.path.insert(0, "/opt/trn_rl_repo")

import ml_dtypes
import numpy as np

import concourse.bass as bass  # noqa: F401
import concourse.mybir as mybir
import concourse.tile as tile
from concourse import bacc
from concourse.bass_utils import run_bass_kernel_spmd

LAST_RESULT = None  # BassKernelResults of the most recent kernel() call

B, Q, K = 16, 64, 1024
QS = KS = VS = 256
H = 128
NCORES = 8
NEG = -1.0e6
QC = 8  # queries per ACT tanh instruction (chunk)
F32 = mybir.dt.float32
BF16 = mybir.dt.bfloat16
NP_BF16 = ml_dtypes.bfloat16


def _build(L, k0, nblk):
    """Build the per-core Bass graph. L/k0/nblk are 2-element lists with the
    per-slot kept key length (multiple of 8), first kept key index, and
    number of 128-key value blocks."""
    nc = bacc.Bacc("TRN2", target_bir_lowering=False, debug=False,
                   num_devices=NCORES)

    inp = {}
    for s in range(2):
        inp[f"keysT{s}"] = nc.dram_tensor(f"keysT{s}", [2, 128, L[s]], BF16,
                                          kind="ExternalInput").ap()
        inp[f"queriesT{s}"] = nc.dram_tensor(f"queriesT{s}", [2, 128, Q], BF16,
                                             kind="ExternalInput").ap()
        inp[f"values{s}"] = nc.dram_tensor(f"values{s}", [nblk[s], 128, VS],
                                           BF16, kind="ExternalInput").ap()
    inp["maskm"] = nc.dram_tensor("maskm", [128, K], F32,
                                  kind="ExternalInput").ap()
    inp["Wk2"] = nc.dram_tensor("Wk2", [2, 128, H], BF16,
                                kind="ExternalInput").ap()
    inp["Wq2"] = nc.dram_tensor("Wq2", [2, 128, H], BF16,
                                kind="ExternalInput").ap()
    inp["ident"] = nc.dram_tensor("ident", [128, 128], F32,
                                  kind="ExternalInput").ap()
    inp["Zmat"] = nc.dram_tensor("Zmat", [128, 256], BF16,
                                 kind="ExternalInput").ap()
    out_d = nc.dram_tensor("out", [128, VS], F32, kind="ExternalOutput").ap()

    with tile.TileContext(nc) as tc:
        with (
            tc.tile_pool(name="consts", bufs=1) as consts,
            tc.tile_pool(name="proj", bufs=1) as proj,
            tc.tile_pool(name="vals", bufs=1) as vals,
            tc.tile_pool(name="tanhbuf", bufs=3) as tanhbuf,
            tc.tile_pool(name="soft", bufs=1) as soft,
        ):
            # constants via GpSimd (SWDGE) so the Sync queue is free for keysT
            ident_sb = consts.tile([128, 128], F32)
            nc.gpsimd.dma_start(out=ident_sb, in_=inp["ident"])
            z_sb = consts.tile([128, 256], BF16)
            nc.gpsimd.dma_start(out=z_sb, in_=inp["Zmat"])
            maskm_sb = consts.tile([128, K], F32)
            nc.gpsimd.dma_start(out=maskm_sb, in_=inp["maskm"])
            wk_sb = consts.tile([128, 2, H], BF16)
            nc.gpsimd.dma_start(out=wk_sb,
                                in_=inp["Wk2"].rearrange("c p h -> p c h"))
            wq_sb = consts.tile([128, 2, H], BF16)
            nc.gpsimd.dma_start(out=wq_sb,
                                in_=inp["Wq2"].rearrange("c p h -> p c h"))

            # ---- phase 1: projections -------------------------------------
            # keysT DMAs are chunked so the kproj matmuls (and hence the
            # first tanh) start as early as possible.
            kfT_sb = []
            qf_sb = []
            with tc.tile_pool(name="kin", bufs=1) as kin, \
                 tc.tile_pool(name="kfps", bufs=2, space="PSUM") as kfps:
                for s in range(2):
                    kT = kin.tile([128, 2, L[s]], BF16, name=f"kT{s}", tag="kT")
                    qT = kin.tile([128, 2, Q], BF16, name=f"qT{s}", tag="qT")
                    nc.gpsimd.dma_start(
                        out=qT, in_=inp[f"queriesT{s}"].rearrange(
                            "c p q -> p c q"))
                    for o in range(0, L[s], 512):
                        w = min(512, L[s] - o)
                        nc.sync.dma_start(
                            out=kT[:, :, o:o + w],
                            in_=inp[f"keysT{s}"].rearrange(
                                "c p l -> p c l")[:, :, o:o + w])

                    kf = proj.tile([128, L[s]], BF16, name=f"kfT{s}",
                                   tag=f"kf{s}")
                    for o in range(0, L[s], 512):
                        w = min(512, L[s] - o)
                        kf_ps = kfps.tile([128, 512], F32, tag="kfps")
                        nc.tensor.matmul(kf_ps[:, :w], wk_sb[:, 0, :],
                                         kT[:, 0, o:o + w], start=True,
                                         stop=False)
                        nc.tensor.matmul(kf_ps[:, :w], wk_sb[:, 1, :],
                                         kT[:, 1, o:o + w], start=False,
                                         stop=True)
                        nc.vector.tensor_copy(out=kf[:, o:o + w],
                                              in_=kf_ps[:, :w])
                    kfT_sb.append(kf)

                    qf_ps = kfps.tile([128, Q], F32, tag="qfps", bufs=1)
                    nc.tensor.matmul(qf_ps, wq_sb[:, 0, :], qT[:, 0, :],
                                     start=True, stop=False)
                    nc.tensor.matmul(qf_ps, wq_sb[:, 1, :], qT[:, 1, :],
                                     start=False, stop=True)
                    qf = proj.tile([128, Q], F32, name=f"qf{s}", tag=f"qf{s}")
                    nc.vector.tensor_copy(out=qf, in_=qf_ps)
                    qf_sb.append(qf)

            # values (needed only in the tail; loads overlap the main loop)
            vals_sb = []
            for s in range(2):
                v = vals.tile([128, nblk[s], VS], BF16, name=f"vals{s}")
                nc.gpsimd.dma_start(
                    out=v, in_=inp[f"values{s}"].rearrange("j p v -> p j v"))
                vals_sb.append(v)

            # ---- phase 2: scores ------------------------------------------
            scps = tc.alloc_tile_pool(name="scps", bufs=1, space="PSUM")
            trps = tc.alloc_tile_pool(name="trps", bufs=2, space="PSUM")
            ops = tc.alloc_tile_pool(name="ops", bufs=1, space="PSUM")
            scA = scps.tile([128, 512], F32, tag="scA")
            scB = scps.tile([128, 512], F32, tag="scB")
            nc.tensor.matmul(scA, ident_sb, maskm_sb[:, 0:512], start=True,
                             stop=False)
            nc.tensor.matmul(scB, ident_sb, maskm_sb[:, 512:1024], start=True,
                             stop=False)

            nchunks = Q // QC
            for s in range(2):
                Ls, k0s = L[s], k0[s]
                nA = 512 - k0s
                for c in range(nchunks):
                    tin = tanhbuf.tile([128, QC * Ls], BF16, tag="tin")
                    for qi in range(QC):
                        q = c * QC + qi
                        nc.vector.tensor_scalar_add(
                            out=tin[:, qi * Ls:(qi + 1) * Ls],
                            in0=kfT_sb[s],
                            scalar1=qf_sb[s][:, q:q + 1])
                    tout = tanhbuf.tile([128, QC * Ls], BF16, tag="tout")
                    nc.scalar.activation(out=tout, in_=tin,
                                         func=mybir.ActivationFunctionType.Tanh)
                    for qi in range(QC):
                        q = c * QC + qi
                        r = s * 64 + q
                        zw = z_sb[:, 128 - r:256 - r]
                        last = (s == 1) and (c == nchunks - 1) and (qi == QC - 1)
                        nc.tensor.matmul(scA[:, k0s:512], zw,
                                         tout[:, qi * Ls:qi * Ls + nA],
                                         start=False, stop=last)
                        nc.tensor.matmul(scB, zw,
                                         tout[:, qi * Ls + nA:qi * Ls + nA + 512],
                                         start=False, stop=last)

            # ---- phase 3: softmax + transpose (unnormalized) --------------
            jmin = min(k0) // 128
            expm = soft.tile([128, K], F32)
            sums = soft.tile([128, 2], F32)
            PT = soft.tile([128, 8 - jmin, 128], BF16)

            nc.scalar.activation(out=expm[:, 0:512], in_=scA,
                                 func=mybir.ActivationFunctionType.Exp,
                                 accum_out=sums[:, 0:1])
            for jj, j in enumerate(range(jmin, 4)):
                tr_ps = trps.tile([128, 128], F32, tag="tr")
                nc.tensor.transpose(tr_ps, expm[:, j * 128:(j + 1) * 128],
                                    ident_sb)
                nc.vector.tensor_copy(out=PT[:, j - jmin, :], in_=tr_ps)
            nc.scalar.activation(out=expm[:, 512:1024], in_=scB,
                                 func=mybir.ActivationFunctionType.Exp,
                                 accum_out=sums[:, 1:2])
            for j in range(4, 8):
                tr_ps = trps.tile([128, 128], F32, tag="tr")
                nc.tensor.transpose(tr_ps, expm[:, j * 128:(j + 1) * 128],
                                    ident_sb)
                nc.vector.tensor_copy(out=PT[:, j - jmin, :], in_=tr_ps)

            stot = soft.tile([128, 1], F32)
            nc.vector.tensor_add(out=stot, in0=sums[:, 0:1], in1=sums[:, 1:2])
            rsum = soft.tile([128, 1], F32)
            nc.vector.reciprocal(out=rsum, in_=stot)

            # ---- phase 4: attn @ values (unnormalized; scaled at the end) --
            o_sb = soft.tile([128, VS], F32)
            for s in range(2):
                js = k0[s] // 128
                out_ps = ops.tile([64, VS], F32, tag=f"out{s}")
                for idx, j in enumerate(range(js, 8)):
                    nc.tensor.matmul(out_ps,
                                     PT[:, j - jmin, s * 64:s * 64 + 64],
                                     vals_sb[s][:, j - js, :],
                                     start=(idx == 0), stop=(j == 7))
                nc.vector.tensor_copy(out=o_sb[s * 64:(s + 1) * 64, :],
                                      in_=out_ps)
            of = soft.tile([128, VS], F32)
            nc.vector.tensor_scalar_mul(out=of, in0=o_sb, scalar1=rsum)
            nc.sync.dma_start(out=out_d, in_=of)
            ops.release()
            trps.release()
            scps.release()

    nc.finalize()
    return nc


def kernel(queries, keys, values, valid_len, Wq, Wk, Ws):
    queries = np.asarray(queries, dtype=np.float32)
    keys = np.asarray(keys, dtype=np.float32)
    values = np.asarray(values, dtype=np.float32)
    Wq = np.asarray(Wq, dtype=np.float32)
    Wk = np.asarray(Wk, dtype=np.float32)
    Ws = np.asarray(Ws, dtype=np.float32)
    vl = np.asarray(valid_len).astype(np.int64)
    assert queries.shape == (B, Q, QS) and keys.shape == (B, K, KS)
    assert values.shape == (B, K, VS) and vl.shape == (B,)

    # Load balance across cores: slot0 takes the 8 most-masked batches
    # (largest valid_len => least work? no: front-mask => keys < vl are
    # masked, so LARGER vl = LESS work). SPMD => per-slot kept length is
    # the max over the slot's batches.
    vlc = np.clip(vl, 0, K - 8)
    order = np.argsort(vlc, kind="stable")  # ascending vl = most work first
    slots = [order[:NCORES], order[NCORES:]]
    k0 = [int(vlc[s].min()) // 8 * 8 for s in slots]
    L = [K - z for z in k0]
    nblk = [8 - z // 128 for z in k0]

    nc = _build(L, k0, nblk)

    # host-side constants
    ident = np.eye(128, dtype=np.float32)
    Zmat = np.zeros((128, 256), dtype=NP_BF16)
    Zmat[:, 128] = Ws.astype(NP_BF16)
    Wk2 = np.ascontiguousarray(Wk.reshape(2, 128, H).astype(NP_BF16))
    Wq2 = np.ascontiguousarray(Wq.reshape(2, 128, H).astype(NP_BF16))

    in_maps = []
    for core in range(NCORES):
        m = {"ident": ident, "Zmat": Zmat, "Wk2": Wk2, "Wq2": Wq2}
        maskm = np.zeros((128, K), dtype=np.float32)
        for s in range(2):
            b = int(slots[s][core])
            m[f"keysT{s}"] = np.ascontiguousarray(
                keys[b, k0[s]:, :].T.reshape(2, 128, L[s]).astype(NP_BF16))
            m[f"queriesT{s}"] = np.ascontiguousarray(
                queries[b].T.reshape(2, 128, Q).astype(NP_BF16))
            m[f"values{s}"] = np.ascontiguousarray(
                values[b, K - nblk[s] * 128:, :].reshape(
                    nblk[s], 128, VS).astype(NP_BF16))
            maskm[s * 64:(s + 1) * 64, :int(vl[b])] = NEG
        m["maskm"] = maskm
        in_maps.append(m)

    res = run_bass_kernel_spmd(nc, in_maps, core_ids=list(range(NCORES)),
                               trace=False)
    global LAST_RESULT
    LAST_RESULT = res

    out = np.empty((B, Q, VS), dtype=np.float32)
    for core in range(NCORES):
        o = res.results[core]["out"]  # [128, VS]
        for s in range(2):
            b = int(slots[s][core])
            out[b] = o[s * 64:(s + 1) * 64, :]
    return out


# revision 19
# speedup vs baseline: 1.9266x; 1.0094x over previous
"""Additive attention (nn_AdditiveAttention) Bass kernel for 8 TRN2 NeuronCores.

Reference computation (B=16, Q=64, K=1024, QS=KS=VS=256, H=128):
    q = queries @ Wq                      # (B,Q,H)
    k = keys @ Wk                         # (B,K,H)
    feat = tanh(q[:,:,None,:] + k[:,None,:,:])   # (B,Q,K,H)
    scores = feat @ Ws                    # (B,Q,K)
    scores = where(arange(K) >= valid_len[b], scores, -1e6)
    out = softmax(scores) @ values        # (B,Q,VS)

Strategy: data-parallel over batch (2 batches per core, "slot0" rows 0-63
and "slot1" rows 64-127 of a 128-row (b,q) partition axis), with
valid_len-aware skipping of masked leading keys (k0 = min valid_len over
the slot, rounded to 8) and bf16 compute on the PE/DVE-heavy stages
(fp32 PE matmul runs at half rate).

Per-core pipeline:
  - kfT[h, k] = (keys @ Wk).T from host-pretransposed bf16 keysT (PE).
  - per q: DVE tensor_scalar add (kfT + qf[:,q]) in bf16 (4x mode); ACT
    tanh on QC queries per instruction (ACT is the floor engine:
    1 elem/lane/cycle @ 1.2 GHz over B*Q*K_kept*H elements).
  - scores accumulate in PSUM fp32 via one bf16 matmul per (q, col tile)
    with a shifted stationary matrix Z (Ws embedded in column 128): row r
    of the [128 bq, 512] PSUM tile receives exactly q=r's scores. The
    accumulation group is seeded by an fp32 mask matmul (identity @
    additive -1e6 mask) implementing the valid_len masking.
  - softmax without max-subtraction (|scores| <= ~10): ACT exp PSUM->SBUF
    with fused row-sum (accum_out). Attention rows are transposed
    UNNORMALIZED (PE transpose per 128-key block, right after the exp of
    that half); normalization is applied to the final [64, VS] outputs
    (slot1's 1/sum vector is moved to partitions 0-63 by a tiny
    SBUF->SBUF DMA).
  - attn @ values: one bf16 matmul per 128-key block against host-sliced
    bf16 values.
"""

import sys

if "/opt/trn_rl_repo" not in sys.path:
    sys.path.insert(0, "/opt/trn_rl_repo")

import ml_dtypes
import numpy as np

import concourse.bass as bass  # noqa: F401
import concourse.mybir as mybir
import concourse.tile as tile
from concourse import bacc
from concourse.bass_utils import run_bass_kernel_spmd

LAST_RESULT = None  # BassKernelResults of the most recent kernel() call

B, Q, K = 16, 64, 1024
QS = KS = VS = 256
H = 128
NCORES = 8
NEG = -1.0e6
QC = 8  # queries per ACT tanh instruction (chunk)
F32 = mybir.dt.float32
BF16 = mybir.dt.bfloat16
NP_BF16 = ml_dtypes.bfloat16


def _build(L, k0, nblk):
    """Build the per-core Bass graph. L/k0/nblk are 2-element lists with the
    per-slot kept key length (multiple of 8), first kept key index, and
    number of 128-key value blocks."""
    nc = bacc.Bacc("TRN2", target_bir_lowering=False, debug=False,
                   num_devices=NCORES)

    inp = {}
    for s in range(2):
        inp[f"keysT{s}"] = nc.dram_tensor(f"keysT{s}", [2, 128, L[s]], BF16,
                                          kind="ExternalInput").ap()
        inp[f"queriesT{s}"] = nc.dram_tensor(f"queriesT{s}", [2, 128, Q], BF16,
                                             kind="ExternalInput").ap()
        inp[f"values{s}"] = nc.dram_tensor(f"values{s}", [nblk[s], 128, VS],
                                           BF16, kind="ExternalInput").ap()
    inp["maskm"] = nc.dram_tensor("maskm", [128, K], F32,
                                  kind="ExternalInput").ap()
    inp["Wk2"] = nc.dram_tensor("Wk2", [2, 128, H], BF16,
                                kind="ExternalInput").ap()
    inp["Wq2"] = nc.dram_tensor("Wq2", [2, 128, H], BF16,
                                kind="ExternalInput").ap()
    inp["ident"] = nc.dram_tensor("ident", [128, 128], F32,
                                  kind="ExternalInput").ap()
    inp["Zmat"] = nc.dram_tensor("Zmat", [128, 256], BF16,
                                 kind="ExternalInput").ap()
    out_d = nc.dram_tensor("out", [128, VS], F32, kind="ExternalOutput").ap()

    with tile.TileContext(nc) as tc:
        with (
            tc.tile_pool(name="consts", bufs=1) as consts,
            tc.tile_pool(name="proj", bufs=1) as proj,
            tc.tile_pool(name="vals", bufs=1) as vals,
            tc.tile_pool(name="tanhbuf", bufs=3) as tanhbuf,
            tc.tile_pool(name="soft", bufs=1) as soft,
        ):
            # constants via GpSimd (SWDGE) so the Sync queue is free for keysT
            ident_sb = consts.tile([128, 128], F32)
            nc.gpsimd.dma_start(out=ident_sb, in_=inp["ident"])
            z_sb = consts.tile([128, 256], BF16)
            nc.gpsimd.dma_start(out=z_sb, in_=inp["Zmat"])
            maskm_sb = consts.tile([128, K], F32)
            nc.gpsimd.dma_start(out=maskm_sb, in_=inp["maskm"])
            wk_sb = consts.tile([128, 2, H], BF16)
            nc.gpsimd.dma_start(out=wk_sb,
                                in_=inp["Wk2"].rearrange("c p h -> p c h"))
            wq_sb = consts.tile([128, 2, H], BF16)
            nc.gpsimd.dma_start(out=wq_sb,
                                in_=inp["Wq2"].rearrange("c p h -> p c h"))

            # ---- phase 1: projections -------------------------------------
            # keysT DMAs are chunked so the kproj matmuls (and hence the
            # first tanh) start as early as possible.
            kfT_sb = []
            qf_sb = []
            with tc.tile_pool(name="kin", bufs=1) as kin, \
                 tc.tile_pool(name="kfps", bufs=2, space="PSUM") as kfps:
                for s in range(2):
                    kT = kin.tile([128, 2, L[s]], BF16, name=f"kT{s}", tag="kT")
                    qT = kin.tile([128, 2, Q], BF16, name=f"qT{s}", tag="qT")
                    nc.gpsimd.dma_start(
                        out=qT, in_=inp[f"queriesT{s}"].rearrange(
                            "c p q -> p c q"))
                    for o in range(0, L[s], 512):
                        w = min(512, L[s] - o)
                        nc.sync.dma_start(
                            out=kT[:, :, o:o + w],
                            in_=inp[f"keysT{s}"].rearrange(
                                "c p l -> p c l")[:, :, o:o + w])

                    kf = proj.tile([128, L[s]], BF16, name=f"kfT{s}",
                                   tag=f"kf{s}")
                    for o in range(0, L[s], 512):
                        w = min(512, L[s] - o)
                        kf_ps = kfps.tile([128, 512], F32, tag="kfps")
                        nc.tensor.matmul(kf_ps[:, :w], wk_sb[:, 0, :],
                                         kT[:, 0, o:o + w], start=True,
                                         stop=False)
                        nc.tensor.matmul(kf_ps[:, :w], wk_sb[:, 1, :],
                                         kT[:, 1, o:o + w], start=False,
                                         stop=True)
                        nc.vector.tensor_copy(out=kf[:, o:o + w],
                                              in_=kf_ps[:, :w])
                    kfT_sb.append(kf)

                    qf_ps = kfps.tile([128, Q], F32, tag="qfps", bufs=1)
                    nc.tensor.matmul(qf_ps, wq_sb[:, 0, :], qT[:, 0, :],
                                     start=True, stop=False)
                    nc.tensor.matmul(qf_ps, wq_sb[:, 1, :], qT[:, 1, :],
                                     start=False, stop=True)
                    qf = proj.tile([128, Q], F32, name=f"qf{s}", tag=f"qf{s}")
                    nc.vector.tensor_copy(out=qf, in_=qf_ps)
                    qf_sb.append(qf)

            # values (needed only in the tail; loads overlap the main loop)
            vals_sb = []
            for s in range(2):
                v = vals.tile([128, nblk[s], VS], BF16, name=f"vals{s}")
                nc.gpsimd.dma_start(
                    out=v, in_=inp[f"values{s}"].rearrange("j p v -> p j v"))
                vals_sb.append(v)

            # ---- phase 2: scores ------------------------------------------
            scps = tc.alloc_tile_pool(name="scps", bufs=1, space="PSUM")
            trps = tc.alloc_tile_pool(name="trps", bufs=2, space="PSUM")
            ops = tc.alloc_tile_pool(name="ops", bufs=1, space="PSUM")
            scA = scps.tile([128, 512], F32, tag="scA")
            scB = scps.tile([128, 512], F32, tag="scB")
            nc.tensor.matmul(scA, ident_sb, maskm_sb[:, 0:512], start=True,
                             stop=False)
            nc.tensor.matmul(scB, ident_sb, maskm_sb[:, 512:1024], start=True,
                             stop=False)

            nchunks = Q // QC
            for s in range(2):
                Ls, k0s = L[s], k0[s]
                nA = 512 - k0s
                for c in range(nchunks):
                    tin = tanhbuf.tile([128, QC * Ls], BF16, tag="tin")
                    for qi in range(QC):
                        q = c * QC + qi
                        if c == 0:
                            # per-k-chunk adds: start as soon as each kfT
                            # chunk lands (shortens the pipeline head)
                            for o in range(0, Ls, 512):
                                w = min(512, Ls - o)
                                nc.vector.tensor_scalar_add(
                                    out=tin[:, qi * Ls + o:qi * Ls + o + w],
                                    in0=kfT_sb[s][:, o:o + w],
                                    scalar1=qf_sb[s][:, q:q + 1])
                        else:
                            nc.vector.tensor_scalar_add(
                                out=tin[:, qi * Ls:(qi + 1) * Ls],
                                in0=kfT_sb[s],
                                scalar1=qf_sb[s][:, q:q + 1])
                    tout = tanhbuf.tile([128, QC * Ls], BF16, tag="tout")
                    nc.scalar.activation(out=tout, in_=tin,
                                         func=mybir.ActivationFunctionType.Tanh)
                    lastc = (s == 1) and (c == nchunks - 1)
                    if not lastc:
                        for qi in range(QC):
                            q = c * QC + qi
                            r = s * 64 + q
                            zw = z_sb[:, 128 - r:256 - r]
                            nc.tensor.matmul(scA[:, k0s:512], zw,
                                             tout[:, qi * Ls:qi * Ls + nA],
                                             start=False, stop=False)
                            nc.tensor.matmul(
                                scB, zw,
                                tout[:, qi * Ls + nA:qi * Ls + nA + 512],
                                start=False, stop=False)
                    else:
                        # final chunk: finish scA first so exp(scA) can
                        # overlap the scB matmuls
                        for qi in range(QC):
                            r = s * 64 + c * QC + qi
                            nc.tensor.matmul(scA[:, k0s:512],
                                             z_sb[:, 128 - r:256 - r],
                                             tout[:, qi * Ls:qi * Ls + nA],
                                             start=False, stop=(qi == QC - 1))
                        for qi in range(QC):
                            r = s * 64 + c * QC + qi
                            nc.tensor.matmul(
                                scB, z_sb[:, 128 - r:256 - r],
                                tout[:, qi * Ls + nA:qi * Ls + nA + 512],
                                start=False, stop=(qi == QC - 1))

            # ---- phase 3: softmax + transpose (unnormalized) --------------
            jmin = min(k0) // 128
            expm = soft.tile([128, K], F32)
            sums = soft.tile([128, 2], F32)
            PT = soft.tile([128, 8 - jmin, 128], BF16)

            out_ps = [ops.tile([64, VS], F32, tag="out0", name="out_ps0"),
                      ops.tile([64, VS], F32, tag="out1", name="out_ps1")]

            def do_half(sc, sumcol, jrange):
                nc.scalar.activation(out=expm[:, jrange[0] * 128:
                                              jrange[-1] * 128 + 128],
                                     in_=sc,
                                     func=mybir.ActivationFunctionType.Exp,
                                     accum_out=sums[:, sumcol:sumcol + 1])
                for j in jrange:
                    tr_ps = trps.tile([128, 128], F32, tag="tr")
                    nc.tensor.transpose(tr_ps, expm[:, j * 128:(j + 1) * 128],
                                        ident_sb)
                    nc.vector.tensor_copy(out=PT[:, j - jmin, :], in_=tr_ps)
                    # attn@V for this key block, both slots (unnormalized)
                    for s in range(2):
                        js = k0[s] // 128
                        if j >= js:
                            nc.tensor.matmul(out_ps[s],
                                             PT[:, j - jmin,
                                                s * 64:s * 64 + 64],
                                             vals_sb[s][:, j - js, :],
                                             start=(j == js), stop=(j == 7))

            do_half(scA, 0, list(range(jmin, 4)))
            do_half(scB, 1, [4, 5, 6, 7])

            stot = soft.tile([128, 1], F32)
            nc.vector.tensor_add(out=stot, in0=sums[:, 0:1], in1=sums[:, 1:2])
            rsum = soft.tile([128, 1], F32)
            nc.vector.reciprocal(out=rsum, in_=stot)

            o_sb = soft.tile([128, VS], F32)
            for s in range(2):
                nc.vector.tensor_copy(out=o_sb[s * 64:(s + 1) * 64, :],
                                      in_=out_ps[s])
            of = soft.tile([128, VS], F32)
            nc.vector.tensor_scalar_mul(out=of, in0=o_sb, scalar1=rsum)
            nc.sync.dma_start(out=out_d, in_=of)
            ops.release()
            trps.release()
            scps.release()

    nc.finalize()
    return nc


def kernel(queries, keys, values, valid_len, Wq, Wk, Ws):
    queries = np.asarray(queries, dtype=np.float32)
    keys = np.asarray(keys, dtype=np.float32)
    values = np.asarray(values, dtype=np.float32)
    Wq = np.asarray(Wq, dtype=np.float32)
    Wk = np.asarray(Wk, dtype=np.float32)
    Ws = np.asarray(Ws, dtype=np.float32)
    vl = np.asarray(valid_len).astype(np.int64)
    assert queries.shape == (B, Q, QS) and keys.shape == (B, K, KS)
    assert values.shape == (B, K, VS) and vl.shape == (B,)

    # Load balance across cores: slot0 takes the 8 most-masked batches
    # (largest valid_len => least work? no: front-mask => keys < vl are
    # masked, so LARGER vl = LESS work). SPMD => per-slot kept length is
    # the max over the slot's batches.
    vlc = np.clip(vl, 0, K - 8)
    order = np.argsort(vlc, kind="stable")  # ascending vl = most work first
    slots = [order[:NCORES], order[NCORES:]]
    k0 = [int(vlc[s].min()) // 8 * 8 for s in slots]
    L = [K - z for z in k0]
    nblk = [8 - z // 128 for z in k0]

    nc = _build(L, k0, nblk)

    # host-side constants
    ident = np.eye(128, dtype=np.float32)
    Zmat = np.zeros((128, 256), dtype=NP_BF16)
    Zmat[:, 128] = Ws.astype(NP_BF16)
    Wk2 = np.ascontiguousarray(Wk.reshape(2, 128, H).astype(NP_BF16))
    Wq2 = np.ascontiguousarray(Wq.reshape(2, 128, H).astype(NP_BF16))

    in_maps = []
    for core in range(NCORES):
        m = {"ident": ident, "Zmat": Zmat, "Wk2": Wk2, "Wq2": Wq2}
        maskm = np.zeros((128, K), dtype=np.float32)
        for s in range(2):
            b = int(slots[s][core])
            m[f"keysT{s}"] = np.ascontiguousarray(
                keys[b, k0[s]:, :].T.reshape(2, 128, L[s]).astype(NP_BF16))
            m[f"queriesT{s}"] = np.ascontiguousarray(
                queries[b].T.reshape(2, 128, Q).astype(NP_BF16))
            m[f"values{s}"] = np.ascontiguousarray(
                values[b, K - nblk[s] * 128:, :].reshape(
                    nblk[s], 128, VS).astype(NP_BF16))
            maskm[s * 64:(s + 1) * 64, :int(vl[b])] = NEG
        m["maskm"] = maskm
        in_maps.append(m)

    res = run_bass_kernel_spmd(nc, in_maps, core_ids=list(range(NCORES)),
                               trace=False)
    global LAST_RESULT
    LAST_RESULT = res

    out = np.empty((B, Q, VS), dtype=np.float32)
    for core in range(NCORES):
        o = res.results[core]["out"]  # [128, VS]
        for s in range(2):
            b = int(slots[s][core])
            out[b] = o[s * 64:(s + 1) * 64, :]
    return out


# revision 23
# speedup vs baseline: 1.9323x; 1.0030x over previous
"""Additive attention (nn_AdditiveAttention) Bass kernel for 8 TRN2 NeuronCores.

Reference computation (B=16, Q=64, K=1024, QS=KS=VS=256, H=128):
    q = queries @ Wq                      # (B,Q,H)
    k = keys @ Wk                         # (B,K,H)
    feat = tanh(q[:,:,None,:] + k[:,None,:,:])   # (B,Q,K,H)
    scores = feat @ Ws                    # (B,Q,K)
    scores = where(arange(K) >= valid_len[b], scores, -1e6)
    out = softmax(scores) @ values        # (B,Q,VS)

Strategy: data-parallel over batch (2 batches per core, "slot0" rows 0-63
and "slot1" rows 64-127 of a 128-row (b,q) partition axis), with
valid_len-aware skipping of masked leading keys (k0 = min valid_len over
the slot, rounded to 8) and bf16 compute on the PE/DVE-heavy stages
(fp32 PE matmul runs at half rate).

Per-core pipeline:
  - kfT[h, k] = (keys @ Wk).T from host-pretransposed bf16 keysT (PE).
  - per q: DVE tensor_scalar add (kfT + qf[:,q]) in bf16 (4x mode); ACT
    tanh on QC queries per instruction (ACT is the floor engine:
    1 elem/lane/cycle @ 1.2 GHz over B*Q*K_kept*H elements).
  - scores accumulate in PSUM fp32 via one bf16 matmul per (q, col tile)
    with a shifted stationary matrix Z (Ws embedded in column 128): row r
    of the [128 bq, 512] PSUM tile receives exactly q=r's scores. The
    accumulation group is seeded by an fp32 mask matmul (identity @
    additive -1e6 mask) implementing the valid_len masking.
  - softmax without max-subtraction (|scores| <= ~10): ACT exp PSUM->SBUF
    with fused row-sum (accum_out). Attention rows are transposed
    UNNORMALIZED (PE transpose per 128-key block, right after the exp of
    that half); normalization is applied to the final [64, VS] outputs
    (slot1's 1/sum vector is moved to partitions 0-63 by a tiny
    SBUF->SBUF DMA).
  - attn @ values: one bf16 matmul per 128-key block against host-sliced
    bf16 values.
"""

import sys

if "/opt/trn_rl_repo" not in sys.path:
    sys.path.insert(0, "/opt/trn_rl_repo")

import ml_dtypes
import numpy as np

import concourse.bass as bass  # noqa: F401
import concourse.mybir as mybir
import concourse.tile as tile
from concourse import bacc
from concourse.bass_utils import run_bass_kernel_spmd

LAST_RESULT = None  # BassKernelResults of the most recent kernel() call

B, Q, K = 16, 64, 1024
QS = KS = VS = 256
H = 128
NCORES = 8
NEG = -1.0e6
QC = 8  # queries per ACT tanh instruction (chunk)
F32 = mybir.dt.float32
BF16 = mybir.dt.bfloat16
NP_BF16 = ml_dtypes.bfloat16


def _build(L, k0, nblk):
    """Build the per-core Bass graph. L/k0/nblk are 2-element lists with the
    per-slot kept key length (multiple of 8), first kept key index, and
    number of 128-key value blocks."""
    nc = bacc.Bacc("TRN2", target_bir_lowering=False, debug=False,
                   num_devices=NCORES)

    inp = {}
    for s in range(2):
        inp[f"keysT{s}"] = nc.dram_tensor(f"keysT{s}", [2, 128, L[s]], BF16,
                                          kind="ExternalInput").ap()
        inp[f"queriesT{s}"] = nc.dram_tensor(f"queriesT{s}", [2, 128, Q], BF16,
                                             kind="ExternalInput").ap()
        inp[f"values{s}"] = nc.dram_tensor(f"values{s}", [nblk[s], 128, VS],
                                           BF16, kind="ExternalInput").ap()
    inp["maskm"] = nc.dram_tensor("maskm", [128, K], F32,
                                  kind="ExternalInput").ap()
    inp["Wk2"] = nc.dram_tensor("Wk2", [2, 128, H], BF16,
                                kind="ExternalInput").ap()
    inp["Wq2"] = nc.dram_tensor("Wq2", [2, 128, H], BF16,
                                kind="ExternalInput").ap()
    inp["ident"] = nc.dram_tensor("ident", [128, 128], F32,
                                  kind="ExternalInput").ap()
    inp["Zmat"] = nc.dram_tensor("Zmat", [128, 256], BF16,
                                 kind="ExternalInput").ap()
    out_d = nc.dram_tensor("out", [128, VS], F32, kind="ExternalOutput").ap()

    with tile.TileContext(nc) as tc:
        with (
            tc.tile_pool(name="consts", bufs=1) as consts,
            tc.tile_pool(name="proj", bufs=1) as proj,
            tc.tile_pool(name="vals", bufs=1) as vals,
            tc.tile_pool(name="tanhbuf", bufs=3) as tanhbuf,
            tc.tile_pool(name="soft", bufs=1) as soft,
        ):
            # constants via GpSimd (SWDGE) so the Sync queue is free for keysT
            ident_sb = consts.tile([128, 128], F32)
            nc.gpsimd.dma_start(out=ident_sb, in_=inp["ident"])
            z_sb = consts.tile([128, 256], BF16)
            nc.gpsimd.dma_start(out=z_sb, in_=inp["Zmat"])
            maskm_sb = consts.tile([128, K], F32)
            nc.scalar.dma_start(out=maskm_sb, in_=inp["maskm"])
            wk_sb = consts.tile([128, 2, H], BF16)
            nc.gpsimd.dma_start(out=wk_sb,
                                in_=inp["Wk2"].rearrange("c p h -> p c h"))
            wq_sb = consts.tile([128, 2, H], BF16)
            nc.gpsimd.dma_start(out=wq_sb,
                                in_=inp["Wq2"].rearrange("c p h -> p c h"))

            # ---- phase 1: projections -------------------------------------
            # keysT DMAs are chunked so the kproj matmuls (and hence the
            # first tanh) start as early as possible.
            kfT_sb = []
            qf_sb = []
            with tc.tile_pool(name="kin", bufs=1) as kin, \
                 tc.tile_pool(name="kfps", bufs=2, space="PSUM") as kfps:
                for s in range(2):
                    kT = kin.tile([128, 2, L[s]], BF16, name=f"kT{s}", tag="kT")
                    qT = kin.tile([128, 2, Q], BF16, name=f"qT{s}", tag="qT")
                    nc.gpsimd.dma_start(
                        out=qT, in_=inp[f"queriesT{s}"].rearrange(
                            "c p q -> p c q"))
                    for ci, o in enumerate(range(0, L[s], 512)):
                        w = min(512, L[s] - o)
                        eng = nc.sync if ci % 2 == 0 else nc.scalar
                        eng.dma_start(
                            out=kT[:, :, o:o + w],
                            in_=inp[f"keysT{s}"].rearrange(
                                "c p l -> p c l")[:, :, o:o + w])

                    kf = proj.tile([128, L[s]], BF16, name=f"kfT{s}",
                                   tag=f"kf{s}")
                    for o in range(0, L[s], 512):
                        w = min(512, L[s] - o)
                        kf_ps = kfps.tile([128, 512], F32, tag="kfps")
                        nc.tensor.matmul(kf_ps[:, :w], wk_sb[:, 0, :],
                                         kT[:, 0, o:o + w], start=True,
                                         stop=False)
                        nc.tensor.matmul(kf_ps[:, :w], wk_sb[:, 1, :],
                                         kT[:, 1, o:o + w], start=False,
                                         stop=True)
                        nc.vector.tensor_copy(out=kf[:, o:o + w],
                                              in_=kf_ps[:, :w])
                    kfT_sb.append(kf)

                    qf_ps = kfps.tile([128, Q], F32, tag="qfps", bufs=1)
                    nc.tensor.matmul(qf_ps, wq_sb[:, 0, :], qT[:, 0, :],
                                     start=True, stop=False)
                    nc.tensor.matmul(qf_ps, wq_sb[:, 1, :], qT[:, 1, :],
                                     start=False, stop=True)
                    qf = proj.tile([128, Q], F32, name=f"qf{s}", tag=f"qf{s}")
                    nc.vector.tensor_copy(out=qf, in_=qf_ps)
                    qf_sb.append(qf)

            # values (needed only in the tail; loads overlap the main loop)
            vals_sb = []
            for s in range(2):
                v = vals.tile([128, nblk[s], VS], BF16, name=f"vals{s}")
                eng = nc.sync if s == 0 else nc.scalar
                eng.dma_start(
                    out=v, in_=inp[f"values{s}"].rearrange("j p v -> p j v"))
                vals_sb.append(v)

            # ---- phase 2: scores ------------------------------------------
            scps = tc.alloc_tile_pool(name="scps", bufs=1, space="PSUM")
            trps = tc.alloc_tile_pool(name="trps", bufs=2, space="PSUM")
            ops = tc.alloc_tile_pool(name="ops", bufs=1, space="PSUM")
            scA = scps.tile([128, 512], F32, tag="scA")
            scB = scps.tile([128, 512], F32, tag="scB")
            nc.tensor.matmul(scA, ident_sb, maskm_sb[:, 0:512], start=True,
                             stop=False)
            nc.tensor.matmul(scB, ident_sb, maskm_sb[:, 512:1024], start=True,
                             stop=False)

            nchunks = Q // QC
            for s in range(2):
                Ls, k0s = L[s], k0[s]
                nA = 512 - k0s
                for c in range(nchunks):
                    tin = tanhbuf.tile([128, QC * Ls], BF16, tag="tin")
                    for qi in range(QC):
                        q = c * QC + qi
                        if c == 0:
                            # per-k-chunk adds: start as soon as each kfT
                            # chunk lands (shortens the pipeline head)
                            for o in range(0, Ls, 512):
                                w = min(512, Ls - o)
                                nc.vector.tensor_scalar_add(
                                    out=tin[:, qi * Ls + o:qi * Ls + o + w],
                                    in0=kfT_sb[s][:, o:o + w],
                                    scalar1=qf_sb[s][:, q:q + 1])
                        else:
                            nc.vector.tensor_scalar_add(
                                out=tin[:, qi * Ls:(qi + 1) * Ls],
                                in0=kfT_sb[s],
                                scalar1=qf_sb[s][:, q:q + 1])
                    tout = tanhbuf.tile([128, QC * Ls], BF16, tag="tout")
                    nc.scalar.activation(out=tout, in_=tin,
                                         func=mybir.ActivationFunctionType.Tanh)
                    lastc = (s == 1) and (c == nchunks - 1)
                    if not lastc:
                        for qi in range(QC):
                            q = c * QC + qi
                            r = s * 64 + q
                            zw = z_sb[:, 128 - r:256 - r]
                            nc.tensor.matmul(scA[:, k0s:512], zw,
                                             tout[:, qi * Ls:qi * Ls + nA],
                                             start=False, stop=False)
                            nc.tensor.matmul(
                                scB, zw,
                                tout[:, qi * Ls + nA:qi * Ls + nA + 512],
                                start=False, stop=False)
                    else:
                        # final chunk: finish scA first so exp(scA) can
                        # overlap the scB matmuls
                        for qi in range(QC):
                            r = s * 64 + c * QC + qi
                            nc.tensor.matmul(scA[:, k0s:512],
                                             z_sb[:, 128 - r:256 - r],
                                             tout[:, qi * Ls:qi * Ls + nA],
                                             start=False, stop=(qi == QC - 1))
                        for qi in range(QC):
                            r = s * 64 + c * QC + qi
                            nc.tensor.matmul(
                                scB, z_sb[:, 128 - r:256 - r],
                                tout[:, qi * Ls + nA:qi * Ls + nA + 512],
                                start=False, stop=(qi == QC - 1))

            # ---- phase 3: softmax + transpose (unnormalized) --------------
            jmin = min(k0) // 128
            expm = soft.tile([128, K], F32)
            sums = soft.tile([128, 2], F32)
            PT = soft.tile([128, 8 - jmin, 128], BF16)

            out_ps = [ops.tile([64, VS], F32, tag="out0", name="out_ps0"),
                      ops.tile([64, VS], F32, tag="out1", name="out_ps1")]

            def do_half(sc, sumcol, jrange):
                nc.scalar.activation(out=expm[:, jrange[0] * 128:
                                              jrange[-1] * 128 + 128],
                                     in_=sc,
                                     func=mybir.ActivationFunctionType.Exp,
                                     accum_out=sums[:, sumcol:sumcol + 1])
                for j in jrange:
                    tr_ps = trps.tile([128, 128], F32, tag="tr")
                    nc.tensor.transpose(tr_ps, expm[:, j * 128:(j + 1) * 128],
                                        ident_sb)
                    nc.vector.tensor_copy(out=PT[:, j - jmin, :], in_=tr_ps)
                    # attn@V for this key block, both slots (unnormalized)
                    for s in range(2):
                        js = k0[s] // 128
                        if j >= js:
                            nc.tensor.matmul(out_ps[s],
                                             PT[:, j - jmin,
                                                s * 64:s * 64 + 64],
                                             vals_sb[s][:, j - js, :],
                                             start=(j == js), stop=(j == 7))

            do_half(scA, 0, list(range(jmin, 4)))
            do_half(scB, 1, [4, 5, 6, 7])

            stot = soft.tile([128, 1], F32)
            nc.vector.tensor_add(out=stot, in0=sums[:, 0:1], in1=sums[:, 1:2])
            rsum = soft.tile([128, 1], F32)
            nc.vector.reciprocal(out=rsum, in_=stot)

            o_sb = soft.tile([128, VS], F32)
            for s in range(2):
                nc.vector.tensor_copy(out=o_sb[s * 64:(s + 1) * 64, :],
                                      in_=out_ps[s])
            of = soft.tile([128, VS], F32)
            nc.vector.tensor_scalar_mul(out=of, in0=o_sb, scalar1=rsum)
            nc.sync.dma_start(out=out_d, in_=of)
            ops.release()
            trps.release()
            scps.release()

    nc.finalize()
    return nc


def kernel(queries, keys, values, valid_len, Wq, Wk, Ws):
    queries = np.asarray(queries, dtype=np.float32)
    keys = np.asarray(keys, dtype=np.float32)
    values = np.asarray(values, dtype=np.float32)
    Wq = np.asarray(Wq, dtype=np.float32)
    Wk = np.asarray(Wk, dtype=np.float32)
    Ws = np.asarray(Ws, dtype=np.float32)
    vl = np.asarray(valid_len).astype(np.int64)
    assert queries.shape == (B, Q, QS) and keys.shape == (B, K, KS)
    assert values.shape == (B, K, VS) and vl.shape == (B,)

    # Load balance across cores: slot0 takes the 8 most-masked batches
    # (largest valid_len => least work? no: front-mask => keys < vl are
    # masked, so LARGER vl = LESS work). SPMD => per-slot kept length is
    # the max over the slot's batches.
    vlc = np.clip(vl, 0, K - 8)
    order = np.argsort(vlc, kind="stable")  # ascending vl = most work first
    slots = [order[:NCORES], order[NCORES:]]
    k0 = [int(vlc[s].min()) // 8 * 8 for s in slots]
    L = [K - z for z in k0]
    nblk = [8 - z // 128 for z in k0]

    nc = _build(L, k0, nblk)

    # host-side constants
    ident = np.eye(128, dtype=np.float32)
    Zmat = np.zeros((128, 256), dtype=NP_BF16)
    Zmat[:, 128] = Ws.astype(NP_BF16)
    Wk2 = np.ascontiguousarray(Wk.reshape(2, 128, H).astype(NP_BF16))
    Wq2 = np.ascontiguousarray(Wq.reshape(2, 128, H).astype(NP_BF16))

    in_maps = []
    for core in range(NCORES):
        m = {"ident": ident, "Zmat": Zmat, "Wk2": Wk2, "Wq2": Wq2}
        maskm = np.zeros((128, K), dtype=np.float32)
        for s in range(2):
            b = int(slots[s][core])
            m[f"keysT{s}"] = np.ascontiguousarray(
                keys[b, k0[s]:, :].T.reshape(2, 128, L[s]).astype(NP_BF16))
            m[f"queriesT{s}"] = np.ascontiguousarray(
                queries[b].T.reshape(2, 128, Q).astype(NP_BF16))
            m[f"values{s}"] = np.ascontiguousarray(
                values[b, K - nblk[s] * 128:, :].reshape(
                    nblk[s], 128, VS).astype(NP_BF16))
            maskm[s * 64:(s + 1) * 64, :int(vl[b])] = NEG
        m["maskm"] = maskm
        in_maps.append(m)

    res = run_bass_kernel_spmd(nc, in_maps, core_ids=list(range(NCORES)),
                               trace=False)
    global LAST_RESULT
    LAST_RESULT = res

    out = np.empty((B, Q, VS), dtype=np.float32)
    for core in range(NCORES):
        o = res.results[core]["out"]  # [128, VS]
        for s in range(2):
            b = int(slots[s][core])
            out[b] = o[s * 64:(s + 1) * 64, :]
    return out
